# revision 3
# baseline (speedup 1.0000x reference)
"""Trainium2 Bass kernel for a 1D-CNN value network (dense_cnn).

Data-parallel over 8 NeuronCores: batch 32768 -> 4096/core.

Design (vs the fp32 per-tile baseline, ~1.6x faster in TimelineSim):
  - bf16 activations end-to-end (fp32 PSUM accumulation); rel err ~1e-2.
  - Residual stream lives in CT layout [128 ch, positions]; the conv2
    output is folded into it with ONE fused scalar_tensor_tensor
    (psum + bias + residual add) -- no separate eviction, no transpose
    back, and the final mean-pool is a single free-dim reduce.
  - CT->TC trips (for LayerNorm stats/normalize, which need positions on
    partitions) go through the DMA xbar transpose (one instruction per
    [128, 6, 128] group); TC->CT trips (conv inputs) stay on the PE
    (bf16 transpose-mode matmuls into bf16 PSUM).
  - LN stats: per-tile bn_stats (HW requires 6-elem output) with bf16
    stats tiles (2x DVE mode), then a grouped even/odd closed-form
    combine (6 small ops per LN instead of 6 bn_aggr), sqrt on the
    scalar engine, reciprocal_approx_fast on DVE.
  - relu+LN-affine fused into scalar-engine PSUM evictions after the PE
    transposes; conv bias fused into the conv1 eviction.
  - S=32 samples per chunk, W=10 chunks in flight, 4-deep PSUM transpose
    pool + double-buffered conv PSUM; head reuses the conv PSUM tag.
  - GPSIMD cannot touch PSUM and has no ALU ops in the backend, so it is
    used for nothing; DVE+ACT split the elementwise work ~evenly.
"""

import numpy as np
from contextlib import ExitStack

import concourse.bass as bass
import concourse.bacc as bacc
import concourse.tile as tile
from concourse import mybir
from concourse.bass_utils import run_bass_kernel_spmd
from concourse.masks import make_identity

F32 = mybir.dt.float32
BF16 = mybir.dt.bfloat16
AF = mybir.ActivationFunctionType
OP = mybir.AluOpType

B, L, CIN, F, NBLK = 32768, 24, 15, 128, 9
NCORES = 8
BC = B // NCORES          # 4096 samples per core
S = 32                    # samples per chunk
NCH = BC // S             # 128 chunks
NPOS = S * L              # 768 positions per chunk
NT = NPOS // 128          # 6 TC tiles per chunk
SSUB = 16                 # samples per conv matmul half
NH = S // SSUB            # 2 halves
NSP = SSUB * L            # 384 = conv matmul free size
EPS = 1e-6
W = 10                    # chunks in flight
NPG = 5                   # padded-buffer parity groups


def build():
    nc = bacc.Bacc("TRN2", target_bir_lowering=False, debug=False, num_devices=1)

    d_board = nc.dram_tensor("board_state", [BC, L, CIN], F32, kind="ExternalInput").ap()
    d_aux = nc.dram_tensor("aux_features", [BC, 6], F32, kind="ExternalInput").ap()
    d_c0w = nc.dram_tensor("conv0_w", [7, CIN, F], F32, kind="ExternalInput").ap()
    d_c0b = nc.dram_tensor("conv0_b", [F], F32, kind="ExternalInput").ap()
    d_l1s = nc.dram_tensor("res_ln1_s", [NBLK, F], F32, kind="ExternalInput").ap()
    d_l1b = nc.dram_tensor("res_ln1_b", [NBLK, F], F32, kind="ExternalInput").ap()
    d_w1 = nc.dram_tensor("res_conv1_w", [NBLK, 3, F, F], F32, kind="ExternalInput").ap()
    d_b1 = nc.dram_tensor("res_conv1_b", [NBLK, F], F32, kind="ExternalInput").ap()
    d_l2s = nc.dram_tensor("res_ln2_s", [NBLK, F], F32, kind="ExternalInput").ap()
    d_l2b = nc.dram_tensor("res_ln2_b", [NBLK, F], F32, kind="ExternalInput").ap()
    d_w2 = nc.dram_tensor("res_conv2_w", [NBLK, 3, F, F], F32, kind="ExternalInput").ap()
    d_b2 = nc.dram_tensor("res_conv2_b", [NBLK, F], F32, kind="ExternalInput").ap()
    d_dw = nc.dram_tensor("dense_w", [F + 6, 64], F32, kind="ExternalInput").ap()
    d_db = nc.dram_tensor("dense_b", [64], F32, kind="ExternalInput").ap()
    d_ow = nc.dram_tensor("out_w", [64, 1], F32, kind="ExternalInput").ap()
    d_ob = nc.dram_tensor("out_b", [1], F32, kind="ExternalInput").ap()
    d_out = nc.dram_tensor("out", [BC, 1], F32, kind="ExternalOutput").ap()

    with tile.TileContext(nc) as tc, ExitStack() as ctx:
        P = ctx.enter_context(tc.tile_pool(name="persist", bufs=1))
        WP = ctx.enter_context(tc.tile_pool(name="wts", bufs=1))
        SB = ctx.enter_context(tc.tile_pool(name="work", bufs=3))
        XB = ctx.enter_context(tc.tile_pool(name="xtiles", bufs=10))
        BD = ctx.enter_context(tc.tile_pool(name="board", bufs=3))
        ST = ctx.enter_context(tc.tile_pool(name="stats", bufs=8))
        PS_TR = ctx.enter_context(tc.tile_pool(name="ps_tr", bufs=4, space="PSUM"))
        PS_MM = ctx.enter_context(tc.tile_pool(name="ps_mm", bufs=2, space="PSUM"))

        # ---- weights / constants to SBUF (staged fp32 -> bf16) ----
        w0 = WP.tile([CIN, 7, F], BF16, tag="w0")
        w1 = WP.tile([F, NBLK, 3, F], BF16, tag="w1")
        w2 = WP.tile([F, NBLK, 3, F], BF16, tag="w2")
        wst = WP.tile([F, 7, F], F32, tag="wst", bufs=2)
        nc.sync.dma_start(wst[0:CIN, :, :], d_c0w.transpose([1, 0, 2]))
        nc.vector.tensor_copy(w0[:], wst[0:CIN, :, :])
        for blk in range(NBLK):
            wst1 = WP.tile([F, 7, F], F32, tag="wst", bufs=2)
            nc.sync.dma_start(wst1[:, 0:3, :], d_w1[blk].transpose([1, 0, 2]))
            nc.vector.tensor_copy(w1[:, blk, :, :], wst1[:, 0:3, :])
            wst2 = WP.tile([F, 7, F], F32, tag="wst", bufs=2)
            nc.sync.dma_start(wst2[:, 0:3, :], d_w2[blk].transpose([1, 0, 2]))
            nc.vector.tensor_copy(w2[:, blk, :, :], wst2[:, 0:3, :])

        def load_cvec(dram, tag, n=NBLK):  # [n,128] -> sbuf [128, n] fp32
            t = WP.tile([F, n], F32, tag=tag)
            nc.sync.dma_start(t[:], dram.transpose([1, 0]))
            return t

        l1s = load_cvec(d_l1s, "l1s")
        l1b = load_cvec(d_l1b, "l1b")
        l2s = load_cvec(d_l2s, "l2s")
        l2b = load_cvec(d_l2b, "l2b")
        c1b = load_cvec(d_b1, "c1b")
        c2b = load_cvec(d_b2, "c2b")
        c0b = WP.tile([F, 1], F32, tag="c0b")
        nc.sync.dma_start(c0b[:], d_c0b.unsqueeze(-1))

        dwa = WP.tile([F, 64], F32, tag="dwa")
        nc.sync.dma_start(dwa[:], d_dw[0:F, :])
        # fold the 1/24 mean-pool into the dense weights (we pool with sum)
        nc.vector.tensor_scalar(dwa[:], dwa[:], 1.0 / L, None, OP.mult)
        dwb = WP.tile([6, 64], F32, tag="dwb")
        nc.sync.dma_start(dwb[:], d_dw[F:F + 6, :])
        dbv = WP.tile([64, 1], F32, tag="dbv")
        nc.sync.dma_start(dbv[:], d_db.unsqueeze(-1))
        owv = WP.tile([64, 1], F32, tag="owv")
        nc.sync.dma_start(owv[:], d_ow)
        obv = WP.tile([1, 1], F32, tag="obv")
        nc.sync.dma_start(obv[:], d_ob.unsqueeze(-1))

        aux_ct = P.tile([6, BC], F32, tag="auxct")
        nc.sync.dma_start(aux_ct[:], d_aux.transpose([1, 0]))

        epst = WP.tile([128, 1], F32, tag="epst")
        nc.vector.memset(epst[:], EPS)
        ident = WP.tile([128, 128], F32, tag="ident")
        make_identity(nc, ident[:])
        identb = WP.tile([128, 128], BF16, tag="identb")
        nc.vector.tensor_copy(identb[:], ident[:])

        pooled = P.tile([F, BC], F32, tag="pooled")
        stage = P.tile([1, BC], F32, tag="stage")

        # padded conv-input buffers; borders stay zero forever
        h1p = [P.tile([F, S, 26], BF16, tag=f"h1p{i}", name=f"h1p{i}") for i in range(NPG)]
        h2p = [P.tile([F, S, 26], BF16, tag=f"h2p{i}", name=f"h2p{i}") for i in range(NPG)]
        x0p = [P.tile([CIN, S, 30], BF16, tag=f"x0p{i}", name=f"x0p{i}") for i in range(NPG)]
        for t in (*h1p, *h2p, *x0p):
            nc.vector.memset(t[:], 0.0)

        board_rows = d_board.rearrange("b l c -> (b l) c")

        def ln_stats(src, tag):
            """src: [128, NT, 128] TC bf16 SBUF. Per-tile bn_stats (HW
            requires 6-elem output), then grouped even/odd combine:
            mu = (me+mo)/2, var = (Me+Mo)/128 + ((me-mo)/2)^2.
            Returns mu [128, NT] f32, rstd [128, NT] f32."""
            bns = ST.tile([128, NT, 6], BF16, tag=f"bns{tag}")
            mu = ST.tile([128, NT], F32, tag=f"mu{tag}")
            dh = ST.tile([128, NT], F32, tag=f"dh{tag}")
            dd = ST.tile([128, NT], F32, tag=f"dd{tag}")
            va = ST.tile([128, NT], F32, tag=f"va{tag}")
            sd = ST.tile([128, NT], F32, tag=f"sd{tag}")
            rstd = ST.tile([128, NT], F32, tag=f"rstd{tag}")
            for t in range(NT):
                nc.vector.bn_stats(bns[:, t, :], src[:, t, :])
            me, mo = bns[:, :, 1], bns[:, :, 4]
            Me, Mo = bns[:, :, 2], bns[:, :, 5]
            hm = dd  # scratch reuse: hm = 0.5*mo
            nc.vector.tensor_scalar(hm[:], mo, 0.5, None, OP.mult)
            nc.vector.scalar_tensor_tensor(mu[:], me, 0.5, hm[:], OP.mult, OP.add)
            nc.vector.scalar_tensor_tensor(dh[:], me, 0.5, hm[:], OP.mult, OP.subtract)
            nc.vector.tensor_tensor(dd[:], dh[:], dh[:], OP.mult)
            nc.vector.tensor_tensor(va[:], Me, Mo, OP.add)
            nc.vector.scalar_tensor_tensor(va[:], va[:], 1.0 / 128.0, dd[:],
                                           OP.mult, OP.add)
            nc.scalar.activation(sd[:], va[:], AF.Sqrt, bias=epst[:, 0:1])
            nc.vector.reciprocal_approx_fast(rstd[:], sd[:])
            return mu, rstd

        def normalize(src, mu, rstd, tag):
            """z[:, t, :] = (src[:, t, :] - mu_t) * rstd_t, bf16 out."""
            z = SB.tile([128, NT, 128], BF16, tag="z", bufs=11)
            for t in range(NT):
                nc.vector.tensor_scalar(
                    z[:, t, :], src[:, t, :],
                    mu[:, t:t + 1], rstd[:, t:t + 1],
                    OP.subtract, OP.mult)
            return z

        def tr_to_ct(z):
            """PE transposes: z [128, NT, 128] TC bf16 -> 2 PSUM halves
            [128, 384] f32 (CT, positions contiguous per half)."""
            ph = PS_TR.tile([128, NH, 384], BF16, tag="tr")
            for h in range(NH):
                for t in range(3):
                    nc.tensor.transpose(
                        ph[:, h, t * 128:(t + 1) * 128], z[:, 3 * h + t, :], identb[:])
            return ph

        def conv3(dst_ps, src_pad, w_sb, blk):
            # dst_ps [128, NH, 512]; src_pad [128, S, 26] bf16
            for h in range(NH):
                for k in range(3):
                    nc.tensor.matmul(
                        dst_ps[:, h, 0:NSP],
                        w_sb[:, blk, k, :],
                        src_pad[:, h * SSUB:(h + 1) * SSUB, k:k + 24],
                        start=(k == 0), stop=(k == 2),
                    )

        def do_conv0(ch):
            pg = ch % NPG
            pos0 = ch * NPOS
            bd = []
            for t in range(NT):
                bt = BD.tile([128, CIN], F32, tag="bd", bufs=16)
                nc.sync.dma_start(bt[:], board_rows[pos0 + t * 128: pos0 + (t + 1) * 128, :])
                bd.append(bt)
            x0t = PS_TR.tile([128, 384], F32, tag="tr")
            x0t2 = PS_TR.tile([128, 384], F32, tag="tr")
            for t in range(3):
                nc.tensor.transpose(x0t[0:CIN, t * 128:(t + 1) * 128], bd[t][:], ident[:])
                nc.tensor.transpose(x0t2[0:CIN, t * 128:(t + 1) * 128], bd[3 + t][:], ident[:])
            nc.scalar.activation(
                x0p[pg][:, 0:SSUB, 3:27],
                x0t[0:CIN, :].rearrange("p (s c) -> p s c", s=SSUB), AF.Copy)
            nc.scalar.activation(
                x0p[pg][:, SSUB:S, 3:27],
                x0t2[0:CIN, :].rearrange("p (s c) -> p s c", s=SSUB), AF.Copy)
            c0 = PS_MM.tile([128, NH, 512], F32, tag="mm")
            for h in range(NH):
                for k in range(7):
                    nc.tensor.matmul(
                        c0[:, h, 0:NSP],
                        w0[:, k, :],
                        x0p[pg][:, h * SSUB:(h + 1) * SSUB, k:k + 24],
                        start=(k == 0), stop=(k == 6),
                    )
            x = SB.tile([128, NPOS], BF16, tag="x", bufs=12)
            nc.scalar.activation(
                x[:].rearrange("p (a b) -> p a b", a=NH),
                c0[:, :, 0:NSP], AF.Relu, bias=c0b[:, 0:1])
            return x

        def p1_ln1(st):
            xt = XB.tile([128, NT, 128], BF16, tag="xt", bufs=11)
            nc.sync.dma_start(xt[:], st["x"][:], transpose=True)
            mu1, rstd1 = ln_stats(xt, "a")
            st["z1"] = normalize(xt, mu1, rstd1, "a")

        def p2_conv1(st, blk):
            pg = st["pg"]
            z1t = tr_to_ct(st["z1"])
            nc.scalar.activation(
                h1p[pg][:, :, 1:25].rearrange("p (h s) c -> p h s c", h=NH),
                z1t[:].rearrange("p h (s c) -> p h s c", s=SSUB), AF.Relu,
                bias=l1b[:, blk:blk + 1], scale=l1s[:, blk:blk + 1])
            g = PS_MM.tile([128, NH, 512], F32, tag="mm")
            conv3(g, h1p[pg], w1, blk)
            gsb = SB.tile([128, NPOS], BF16, tag="gsb", bufs=11)
            nc.scalar.activation(
                gsb[:].rearrange("p (a b) -> p a b", a=NH),
                g[:, :, 0:NSP], AF.Identity, bias=c1b[:, blk:blk + 1])
            gt = XB.tile([128, NT, 128], BF16, tag="gt", bufs=11)
            nc.sync.dma_start(gt[:], gsb[:], transpose=True)
            st["gt"] = gt

        def p3_ln2(st):
            mu2, rstd2 = ln_stats(st["gt"], "b")
            st["z2"] = normalize(st["gt"], mu2, rstd2, "b")

        def p4_conv2(st, blk):
            pg = st["pg"]
            z2t = tr_to_ct(st["z2"])
            nc.scalar.activation(
                h2p[pg][:, :, 1:25].rearrange("p (h s) c -> p h s c", h=NH),
                z2t[:].rearrange("p h (s c) -> p h s c", s=SSUB), AF.Relu,
                bias=l2b[:, blk:blk + 1], scale=l2s[:, blk:blk + 1])
            p2 = PS_MM.tile([128, NH, 512], F32, tag="mm")
            conv3(p2, h2p[pg], w2, blk)
            p2b = SB.tile([128, NPOS], BF16, tag="p2b", bufs=11)
            nc.scalar.activation(
                p2b[:].rearrange("p (a b) -> p a b", a=NH),
                p2[:, :, 0:NSP], AF.Identity, bias=c2b[:, blk:blk + 1])
            xnew = SB.tile([128, NPOS], BF16, tag="x", bufs=12)
            nc.vector.tensor_tensor(xnew[:], st["x"][:], p2b[:], OP.add)
            st["x"] = xnew

        def do_pool(ch, x):
            nc.vector.tensor_reduce(
                pooled[:, ch * S:(ch + 1) * S],
                x[:].rearrange("p (s l) -> p s l", l=L),
                mybir.AxisListType.X, OP.add)

        for i in range(0, NCH, W):
            chs = list(range(i, min(i + W, NCH)))
            states = {}
            for c in chs:
                states[c] = {"x": do_conv0(c), "pg": c % NPG}
            for blk in range(NBLK):
                for c in chs:
                    p1_ln1(states[c])
                for c in chs:
                    p2_conv1(states[c], blk)
                for c in chs:
                    p3_ln2(states[c])
                for c in chs:
                    p4_conv2(states[c], blk)
            for c in chs:
                do_pool(c, states[c]["x"])

        # ---------- head ----------
        for j in range(BC // 512):
            hd = PS_MM.tile([128, NH, 512], F32, tag="mm")
            hps = hd[0:64, 0, :]
            nc.tensor.matmul(hps, dwa[:], pooled[:, j * 512:(j + 1) * 512],
                             start=True, stop=False)
            nc.tensor.matmul(hps, dwb[:], aux_ct[:, j * 512:(j + 1) * 512],
                             start=False, stop=True)
            hh = SB.tile([64, 512], F32, tag="hh", bufs=2)
            nc.scalar.activation(hh[:], hps, AF.Relu, bias=dbv[:, 0:1])
            ops = hd[64:65, 0, :]
            nc.tensor.matmul(ops, owv[:], hh[:], start=True, stop=True)
            nc.scalar.activation(stage[0:1, j * 512:(j + 1) * 512], ops,
                                 AF.Tanh, bias=obv[:, 0:1])
        nc.vector.tensor_scalar(stage[:], stage[:], 3.0, None, OP.mult)
        nc.sync.dma_start(d_out.rearrange("b o -> (b o)").unsqueeze(0), stage[:])

    nc.compile()
    return nc


_NC = None


def kernel(**inputs):
    global _NC
    if _NC is None:
        _NC = build()
    full = {k: np.ascontiguousarray(v, dtype=np.float32) for k, v in inputs.items()}
    in_maps = []
    for i in range(NCORES):
        m = {}
        for k, v in full.items():
            if k in ("board_state", "aux_features"):
                m[k] = np.ascontiguousarray(v[i * BC:(i + 1) * BC])
            else:
                m[k] = v
        in_maps.append(m)
    res = run_bass_kernel_spmd(_NC, in_maps, core_ids=list(range(NCORES)))
    return np.concatenate([res.results[i]["out"] for i in range(NCORES)], axis=0)


if __name__ == "__main__":
    rng = np.random.default_rng(0)
    ins = {
        "board_state": rng.standard_normal((B, L, CIN), dtype=np.float32),
        "aux_features": rng.standard_normal((B, 6), dtype=np.float32),
        "conv0_w": rng.standard_normal((7, CIN, F), dtype=np.float32) * 0.05,
        "conv0_b": np.zeros((F,), np.float32),
        "res_ln1_s": np.ones((NBLK, F), np.float32),
        "res_ln1_b": np.zeros((NBLK, F), np.float32),
        "res_conv1_w": rng.standard_normal((NBLK, 3, F, F), dtype=np.float32) * 0.05,
        "res_conv1_b": np.zeros((NBLK, F), np.float32),
        "res_ln2_s": np.ones((NBLK, F), np.float32),
        "res_ln2_b": np.zeros((NBLK, F), np.float32),
        "res_conv2_w": rng.standard_normal((NBLK, 3, F, F), dtype=np.float32) * 0.05,
        "res_conv2_b": np.zeros((NBLK, F), np.float32),
        "dense_w": rng.standard_normal((F + 6, 64), dtype=np.float32) * 0.05,
        "dense_b": np.zeros((64,), np.float32),
        "out_w": rng.standard_normal((64, 1), dtype=np.float32) * 0.05,
        "out_b": np.zeros((1,), np.float32),
    }
    out = kernel(**ins)
    print(out.shape, out[:4, 0])
